# revision 51
# baseline (speedup 1.0000x reference)
"""AttentionNoPairBias on 8 Trainium2 NeuronCores.

Reference computation (B=1, S=N=2048, C=1024, H=16, DH=64), fp32:
    q = s @ Wq.T + bq ; k = k_in @ Wk.T ; v = k_in @ Wv.T
    g = sigmoid(s @ Wg.T)
    attn = softmax(q k^T / sqrt(DH) + (1-mask)*(-1e6))   per head
    out  = (g * (attn @ v)) @ Wo.T

Sharding: tensor-parallel over heads. Core c owns channels
[128c, 128(c+1)) = heads {2c, 2c+1}. Each core:
  - projects q/k/v/g for its 128 channels (contraction over full C,
    activations fed in transposed [C, S] layout so C sits on partitions;
    V is additionally produced directly in natural [key, chan] layout by
    swapping the matmul operands, so no on-chip transposes are needed),
  - runs attention for its 2 heads entirely on-chip
    (logits computed transposed [keys, queries] so the key mask is a
    per-partition bias folded into the Exp activation; softmax
    denominators come from ones-blocks appended to V in the PV matmul),
  - gates with g, writes y.T = (g*o).T to DRAM in 128-query blocks,
  - two AllToAlls reshard y over the sequence dim: the first covers
    queries 0:1024 and is issued after the second S-block, so the
    collective and the first half of the output projection overlap the
    remaining attention blocks,
  - computes out rows {128c..128c+127, 1024+128c..1024+128c+127} with
    the full Wo.
Host reassembles the 16 row-slices.

Mask sparsity: masked keys receive softmax weight exp(-1e6) = 0 in the
reference, so the host drops them up front — k_in is compacted to the
kept keys, padded to a multiple of 256 (pad slots carry the -1e6 bias,
contributing exactly 0 to both numerator and denominator). With the
~50%-dense random mask this cuts the k/v projections, QK, exp and PV
work by ~40% while remaining bit-equivalent to masking. The kernel is
compiled per (n2, jc_att) configuration and cached.

All matmul operands are bf16 (same PE rate as float32r on TRN2 but half
the HBM/SBUF traffic); accumulation, softmax and elementwise math stay
fp32. Measured rel err ~6e-3 vs the fp32 reference (gate 2e-2).
Degenerate all-masked masks (sum == 0, probability ~2^-2048 under the
spec's randint fill) would divide by zero and are not handled.
"""

import numpy as np
import ml_dtypes

B, S, N, C, H = 1, 2048, 2048, 1024, 16
DH = C // H  # 64
P = 128
NCORES = 8
CH = C // NCORES  # 128 channels per core (2 heads)
KC = C // P  # 8 contraction chunks
SBW = 512  # S-block width
NSB = S // SBW  # 4 S blocks
SROWS = S // NCORES  # 256 output rows per core
NEG = -1.0e6
BF = ml_dtypes.bfloat16
NCOLL = 2  # 1: one AllToAll (256-query blocks); 2: split into two


def _build(n2=N, jc_att=None, repeat=1, single=False, nocc=False,
           tinycc=0):
    # n2: compacted key count (multiple of 256, <= N); jc_att: number of
    # 128-key chunks attention actually visits (trailing all-pad chunks
    # contribute exactly 0 and are skipped)
    jc_n = jc_att if jc_att is not None else n2 // P
    nbk = n2 // 256
    import concourse.mybir as mybir
    import concourse.tile as tile
    from concourse import bacc

    f32 = mybir.dt.float32
    bf16 = mybir.dt.bfloat16
    AF = mybir.ActivationFunctionType

    nc = bacc.Bacc("TRN2", target_bir_lowering=False, debug=False,
                   num_devices=(1 if single else NCORES))

    sT = nc.declare_dram_parameter("sT", [P, KC, S], bf16, isOutput=False)
    kT = nc.declare_dram_parameter("kT", [P, KC, n2], bf16, isOutput=False)
    wq = nc.declare_dram_parameter("wq", [P, KC, CH], bf16, isOutput=False)
    wk = nc.declare_dram_parameter("wk", [P, KC, CH], bf16, isOutput=False)
    wv = nc.declare_dram_parameter("wv", [P, KC, CH], bf16, isOutput=False)
    wg = nc.declare_dram_parameter("wg", [P, KC, CH], bf16, isOutput=False)
    bq = nc.declare_dram_parameter("bq", [CH, 1], f32, isOutput=False)
    mb = nc.declare_dram_parameter("mb", [P, jc_n], f32, isOutput=False)
    wo = nc.declare_dram_parameter("wo", [P, KC, C], bf16, isOutput=False)
    out_ext = nc.declare_dram_parameter("out", [SROWS, C], f32, isOutput=True)

    with tile.TileContext(nc) as tc:
        with (
            tc.tile_pool(name="dram", bufs=1, space="DRAM") as dpool,
            tc.tile_pool(name="const", bufs=1) as cpool,
            tc.tile_pool(name="inp", bufs=1) as spool,
            tc.tile_pool(name="kv", bufs=2) as kvpool,
            tc.tile_pool(name="qg", bufs=2) as qgpool,
            tc.tile_pool(name="pp", bufs=8) as ppool,
            tc.tile_pool(name="yy", bufs=2) as ypool,
            tc.tile_pool(name="fin", bufs=1) as fpool,
            tc.tile_pool(name="psA", bufs=3, space="PSUM") as psA,
            tc.tile_pool(name="psB", bufs=2, space="PSUM") as psB,
        ):
            ncc = NCOLL
            ccw = P if ncc == 2 else SROWS  # query block width per rank
            cc_in = [dpool.tile([NCORES, P, ccw], bf16, tag=f"cc{i}in",
                                name=f"cc{i}in") for i in range(ncc)]
            cc_out = [dpool.tile([NCORES, P, ccw], bf16, tag=f"cc{i}out",
                                 name=f"cc{i}out") for i in range(ncc)]
            if tinycc:
                tcc_in = [dpool.tile([NCORES, tinycc, P], bf16,
                                     tag=f"tcc{i}in", name=f"tcc{i}in")
                          for i in range(2)]
                tcc_out = [dpool.tile([NCORES, tinycc, P], bf16,
                                      tag=f"tcc{i}out", name=f"tcc{i}out")
                           for i in range(2)]

            # ---- constants / weights / inputs (issue order = priority) ---
            kTs = spool.tile([P, KC, n2], bf16, tag="kT")
            s_sb = spool.tile([P, KC, S], bf16, tag="sT")
            halfk = KC // 2

            def load_kt(eng):
                eng.dma_start(kTs[:, 0:halfk, 0:256], kT[:, 0:halfk, 0:256])
                eng.dma_start(kTs[:, halfk:KC, 0:256], kT[:, halfk:KC, 0:256])
                for jb in range(1, nbk):
                    eng.dma_start(kTs[:, :, jb * 256:(jb + 1) * 256],
                                  kT[:, :, jb * 256:(jb + 1) * 256])

            def load_s(eng, lo=0, hi=NSB):
                if lo == 0:
                    eng.dma_start(s_sb[:, 0:halfk, 0:SBW],
                                  sT[:, 0:halfk, 0:SBW])
                    eng.dma_start(s_sb[:, halfk:KC, 0:SBW],
                                  sT[:, halfk:KC, 0:SBW])
                for sb in range(max(lo, 1), hi):
                    eng.dma_start(s_sb[:, :, sb * SBW:(sb + 1) * SBW],
                                  sT[:, :, sb * SBW:(sb + 1) * SBW])

            def load_inputs(first_rep):
                # First repeat: a single queue (SP) in exact need order, so
                # the serially-modeled DMA engines deliver phase-A inputs
                # first (a second queue's dispatches would interleave its
                # transfers into the critical stream). Later repeats
                # prefetch from inside the previous repeat instead (see the
                # sb == 3 branch of the attention loop).
                t = cpool.tile([P, KC, CH], bf16, tag="wk")
                nc.sync.dma_start(t[:, 0:halfk, :], wk[:, 0:halfk, :])
                w_sb["wk"] = t
                nc.sync.dma_start(kTs[:, 0:halfk, 0:256],
                                  kT[:, 0:halfk, 0:256])
                nc.sync.dma_start(t[:, halfk:KC, :], wk[:, halfk:KC, :])
                nc.sync.dma_start(kTs[:, halfk:KC, 0:256],
                                  kT[:, halfk:KC, 0:256])
                t = cpool.tile([P, KC, CH], bf16, tag="wv")
                nc.sync.dma_start(t[:], wv[:])
                w_sb["wv"] = t
                for jb in range(1, nbk):
                    nc.sync.dma_start(kTs[:, :, jb * 256:(jb + 1) * 256],
                                      kT[:, :, jb * 256:(jb + 1) * 256])
                load_s(nc.sync, 0, 1)
                bq_sb = cpool.tile([CH, 1], f32, tag="bq")
                nc.sync.dma_start(bq_sb[:], bq[:])
                mb_t = cpool.tile([P, jc_n], f32, tag="mb")
                nc.sync.dma_start(mb_t[:], mb[:])
                ones_t = cpool.tile([P, 1], f32, tag="ones")
                nc.vector.memset(ones_t[:], 1.0)
                actwarm = cpool.tile([P, 1], f32, tag="actwarm")
                nc.scalar.activation(actwarm[:], ones_t[:], AF.Exp)
                for name, ext in (("wq", wq), ("wg", wg)):
                    t = cpool.tile([P, KC, CH], bf16, tag=name)
                    nc.sync.dma_start(t[:], ext[:])
                    w_sb[name] = t
                consts.extend([bq_sb, mb_t, ones_t])
                nc.sync.dma_start(wo_sb[:], wo[:])

            w_sb = {}
            consts = []
            wo_sb = fpool.tile([P, KC, C], bf16, tag="wo")
            aprev = {}

            def do_d():
                # output projection for the PREVIOUS repeat's attention —
                # runs while this repeat's attention is still in flight, so
                # the collectives' latency hides across the rep boundary
                if ncc == 2:
                    osb0 = out_proj(0, aprev[0])
                    nc.sync.dma_start(out_ext[0:P, :], osb0[:])
                    osb1 = out_proj(1, aprev[1])
                    nc.scalar.dma_start(out_ext[P:2 * P, :], osb1[:])
                else:
                    osb0 = out_proj(0, aprev[1], m=0)
                    nc.sync.dma_start(out_ext[0:P, :], osb0[:])
                    osb1 = out_proj(0, aprev[1], m=1)
                    nc.scalar.dma_start(out_ext[P:2 * P, :], osb1[:])

            for _rep in range(repeat):
                if _rep == 0:
                    load_inputs(True)
                bq_sb, mb_sb, ones_c = consts[0], consts[1], consts[2]
                if _rep > 0:
                    aprev[1] = fetch_a(1 if ncc == 2 else 0)

                # ---- phase A: k/v projections --------------------------
                # kt: [chan, key] layout for QK; v: natural [key, chan]
                # with ones blocks for the softmax denominators:
                #   head0: [v0 | ones], head1: [ones | v1]  (128+128 cols)
                kt_sb = kvpool.tile([CH, n2], bf16, tag="kt")
                v_sb = kvpool.tile([P, jc_n, 2 * P], bf16, tag="vn")
                nc.vector.tensor_copy(
                    v_sb[:, :, DH:P],
                    ones_c[:, None, :].to_broadcast([P, jc_n, P - DH]))
                nc.vector.tensor_copy(
                    v_sb[:, :, P:P + DH],
                    ones_c[:, None, :].to_broadcast([P, jc_n, DH]))

                for jb in range(nbk):
                    mm = psA.tile([P, 2 * SBW], f32, tag="mm")
                    for kc in range(KC):
                        nc.tensor.matmul(mm[:, 0:256], w_sb["wk"][:, kc, :],
                                         kTs[:, kc, jb * 256:(jb + 1) * 256],
                                         start=(kc == 0), stop=(kc == KC - 1))
                    for sub in range(2):
                        jcw = 2 * jb + sub
                        if jcw >= jc_n:
                            continue
                        lo = 512 + sub * P
                        for kc in range(KC):
                            nc.tensor.matmul(
                                mm[:, lo:lo + P],
                                kTs[:, kc, jcw * P:(jcw + 1) * P],
                                w_sb["wv"][:, kc, :],
                                start=(kc == 0), stop=(kc == KC - 1))
                    nc.vector.tensor_copy(kt_sb[:, jb * 256:(jb + 1) * 256],
                                          mm[:, 0:256])
                    for sub in range(2):
                        jcw = 2 * jb + sub
                        if jcw >= jc_n:
                            continue
                        lo = 512 + sub * P
                        # head0 v -> cols 0:64 ; head1 v -> cols 192:256
                        nc.vector.tensor_copy(v_sb[:, jcw, 0:DH],
                                              mm[:, lo:lo + DH])
                        nc.vector.tensor_copy(v_sb[:, jcw, P + DH:2 * P],
                                              mm[:, lo + DH:lo + P])

                if _rep > 0:
                    do_d()

                # ---- phase B: q/g proj + attention, per S block ----------
                def qg_proj(sb):
                    qg = psA.tile([P, 2 * SBW], f32, tag="mm")
                    for kc in range(KC):
                        nc.tensor.matmul(qg[:, 0:SBW], w_sb["wq"][:, kc, :],
                                         s_sb[:, kc, sb * SBW:(sb + 1) * SBW],
                                         start=(kc == 0), stop=(kc == KC - 1))
                    for kc in range(KC):
                        nc.tensor.matmul(qg[:, SBW:2 * SBW],
                                         w_sb["wg"][:, kc, :],
                                         s_sb[:, kc, sb * SBW:(sb + 1) * SBW],
                                         start=(kc == 0), stop=(kc == KC - 1))
                    qt = qgpool.tile([CH, SBW], bf16, tag="qt")
                    nc.vector.tensor_add(qt[:], qg[:, 0:SBW],
                                         bq_sb[:].to_broadcast([CH, SBW]))
                    gt = qgpool.tile([CH, SBW], f32, tag="gt")
                    ge = qgpool.tile([CH, SBW], f32, tag="ge")
                    nc.scalar.activation(ge[:], qg[:, SBW:2 * SBW], AF.Exp,
                                         scale=-1.0)
                    nc.vector.tensor_scalar_add(ge[:], ge[:], 1.0)
                    nc.vector.reciprocal(gt[:], ge[:])
                    return qt, gt

                def qk_mm(qt_, jc):
                    qk = psA.tile([P, 2 * SBW], f32, tag="mm")
                    for h in range(2):
                        nc.tensor.matmul(
                            qk[:, h * SBW:(h + 1) * SBW],
                            kt_sb[h * DH:(h + 1) * DH, jc * P:(jc + 1) * P],
                            qt_[h * DH:(h + 1) * DH, :],
                            start=True, stop=True)
                    return qk

                def fetch_a(part):
                    # split across the two HWDGE queues: the Act halves are
                    # only issued at points where the Act queue has no exp
                    # work left behind them, so their waits can't stall it
                    a_sb = fpool.tile([P, KC, ccw], bf16, tag=f"a2a{part}",
                                      name=f"a2a{part}")
                    for kc in range(KC):
                        eng = nc.sync if kc % 2 == 0 else nc.scalar
                        eng.dma_start(a_sb[:, kc, :], cc_out[part][kc, :, :])
                    return a_sb

                def out_proj(part, a_sb, m=0):
                    # 128 output rows with the full Wo, from the part'th
                    # AllToAll's result (block column m when ncc == 1)
                    op = psA.tile([P, 2 * SBW], f32, tag="mm")
                    for nb in range(2):
                        for kc in range(KC):
                            nc.tensor.matmul(
                                op[:, nb * SBW:(nb + 1) * SBW],
                                a_sb[:, kc, m * P:(m + 1) * P],
                                wo_sb[:, kc, nb * SBW:(nb + 1) * SBW],
                                start=(kc == 0), stop=(kc == KC - 1))
                    o_sb = ypool.tile([P, 2 * SBW], f32, tag="osb")
                    for nb in range(2):
                        nc.vector.tensor_copy(o_sb[:, nb * SBW:(nb + 1) * SBW],
                                              op[:, nb * SBW:(nb + 1) * SBW])
                    return o_sb

                def collective(part):
                    if tinycc and not single:
                        nc.sync.dma_start(cc_out[part][:], cc_in[part][:])
                        nc.gpsimd.collective_compute(
                            "AllToAll", mybir.AluOpType.bypass,
                            replica_groups=[list(range(NCORES))],
                            ins=[tcc_in[part][:].opt()],
                            outs=[tcc_out[part][:].opt()])
                    elif single or nocc:
                        nc.sync.dma_start(cc_out[part][:], cc_in[part][:])
                    else:
                        nc.gpsimd.collective_compute(
                            "AllToAll", mybir.AluOpType.bypass,
                            replica_groups=[list(range(NCORES))],
                            ins=[cc_in[part].opt()], outs=[cc_out[part].opt()])

                nxt = qg_proj(0)
                qk = None
                for sb in range(NSB):
                    qt, gt = nxt
                    if sb == 3:
                        if ncc == 2:
                            # fetch the first AllToAll's result now (it
                            # landed mid-attention); consumed by the next
                            # repeat's do_d
                            aprev[0] = fetch_a(0)
                        if _rep + 1 < repeat:
                            # stream the next repeat's inputs now — their
                            # WAR hazards (this repeat's phase A / qg
                            # reads) have all passed, so phase A restarts
                            # without waiting on input DMA
                            load_inputs(False)

                    # attention for the 2 heads; PV accumulates over chunks
                    pv0 = psB.tile([P, SBW], f32, tag="pv")
                    pv1 = psB.tile([P, SBW], f32, tag="pv")
                    pvs = (pv0, pv1)
                    if qk is None:
                        qk = qk_mm(qt, 0)
                    for jc in range(jc_n):
                        pt = ppool.tile([P, 2 * SBW], bf16, tag="pt")
                        nc.scalar.activation(pt[:], qk[:], AF.Exp,
                                             bias=mb_sb[:, jc:jc + 1],
                                             scale=1.0 / np.sqrt(DH))
                        if jc + 1 < jc_n:
                            qk = qk_mm(qt, jc + 1)
                        elif sb + 1 < NSB:
                            # cross-block lookahead: next block's first QK
                            # fills while this block's tail drains
                            qk = qk_mm(nxt[0], 0)
                        else:
                            qk = None
                        for h in range(2):
                            nc.tensor.matmul(
                                pvs[h][:],
                                v_sb[:, jc, h * P:(h + 1) * P],
                                pt[:, h * SBW:(h + 1) * SBW],
                                start=(jc == 0), stop=(jc == jc_n - 1))
                        if jc == min(3, jc_n - 1) and sb + 1 < NSB:
                            # project the next block's q/g now so the next
                            # attention block starts without an ACT bubble
                            nxt = qg_proj(sb + 1)

                    # denominators head straight from PSUM to their o-aligned
                    # partitions while the o values are copied off
                    # (releasing the banks for the next S block's PVs);
                    # y = g*o computed first so only the final mul waits on
                    # the denominator roundtrip.
                    # head0: o@rows0:64 den@64:128, head1: den@0:64 o@64:128
                    o0 = ypool.tile([P, SBW], f32, tag="o0")
                    o1 = ypool.tile([P, SBW], f32, tag="o1")
                    nc.vector.tensor_copy(o0[:], pv0[:])
                    nc.vector.tensor_copy(o1[:], pv1[:])
                    # on the last block the Act queue has no exps left, so
                    # half the latency-critical tail DMAs can dispatch there
                    deng = nc.scalar if sb == NSB - 1 else nc.sync
                    den = ypool.tile([CH, SBW], f32, tag="den")
                    nc.sync.dma_start(den[0:DH, :], o0[DH:P, :])
                    deng.dma_start(den[DH:2 * DH, :], o1[0:DH, :])
                    rec = ypool.tile([CH, SBW], f32, tag="rec")
                    nc.vector.reciprocal(rec[:], den[:])
                    yt = ypool.tile([CH, SBW], bf16, tag="yt")
                    ytmp = ypool.tile([CH, SBW], f32, tag="ytmp")
                    nc.vector.tensor_mul(ytmp[0:DH, :], o0[0:DH, :],
                                         gt[0:DH, :])
                    nc.vector.tensor_mul(ytmp[DH:2 * DH, :], o1[P - DH:P, :],
                                         gt[DH:2 * DH, :])
                    nc.vector.tensor_mul(yt[:], ytmp[:], rec[:])
                    if ncc == 2:
                        part = sb // 2
                        base = (sb % 2) * NSB
                        for qb in range(NSB):
                            eng = deng if qb % 2 == 1 else nc.sync
                            eng.dma_start(cc_in[part][base + qb, :, :],
                                          yt[:, qb * P:(qb + 1) * P])
                        if sb == 1:
                            collective(0)
                        elif sb == 3:
                            collective(1)
                    else:
                        for half in range(2):
                            eng = deng if half == 1 else nc.sync
                            eng.dma_start(
                                cc_in[0][2 * sb + half, :, :],
                                yt[:, half * SROWS:(half + 1) * SROWS])
                        if sb == 3:
                            collective(0)

            # the last repeat's output projection
            aprev[1] = fetch_a(1 if ncc == 2 else 0)
            do_d()

    nc.compile()
    return nc


_NC_CACHE = {}


def _n2_for(mask):
    k0 = int(np.asarray(mask).reshape(-1).astype(np.int64).sum())
    n2 = min(N, max(256, int(np.ceil(max(k0, 1) / 256.0)) * 256))
    jc_att = min(n2 // P, max(1, int(np.ceil(max(k0, 1) / P))))
    return n2, jc_att


def _get_nc(n2, jc_att):
    key = (n2, jc_att)
    if key not in _NC_CACHE:
        _NC_CACHE[key] = _build(n2=n2, jc_att=jc_att)
    return _NC_CACHE[key]


def _in_maps(inputs):
    s = np.asarray(inputs["s"], dtype=np.float32)
    mask = np.asarray(inputs["mask"])
    k_in = np.asarray(inputs["k_in"], dtype=np.float32)
    Wq = np.asarray(inputs["Wq"], dtype=np.float32)
    bqv = np.asarray(inputs["bq"], dtype=np.float32)
    Wk = np.asarray(inputs["Wk"], dtype=np.float32)
    Wv = np.asarray(inputs["Wv"], dtype=np.float32)
    Wg = np.asarray(inputs["Wg"], dtype=np.float32)
    Wo = np.asarray(inputs["Wo"], dtype=np.float32)

    def tileT(x2d):  # [L, C] -> [P, KC, L] with [p,kc,l] = x2d[l, kc*P+p]
        return np.ascontiguousarray(
            x2d.reshape(-1, KC, P).transpose(2, 1, 0)).astype(BF)

    def tilew(w2d):  # [C, CH] -> [P, KC, CH]
        return np.ascontiguousarray(
            w2d.reshape(KC, P, -1).transpose(1, 0, 2)).astype(BF)

    sT = tileT(s[0])

    # compact keys: keep unmasked rows, pad to a multiple of 256 with
    # slots whose bias is -1e6 (their softmax weight is exactly 0)
    n2, jc_att = _n2_for(mask)
    idx = np.flatnonzero(mask[0] != 0)[:n2]
    idx_pad = np.zeros(n2, dtype=np.int64)
    idx_pad[:len(idx)] = idx
    kT = tileT(np.ascontiguousarray(k_in[0][idx_pad]))
    mbias = np.full(jc_att * P, NEG, dtype=np.float32)
    mbias[:len(idx)] = 0.0
    mb_t = np.ascontiguousarray(mbias.reshape(jc_att, P).T)
    woT = np.ascontiguousarray(Wo.T)

    maps = []
    for c in range(NCORES):
        sl = slice(c * CH, (c + 1) * CH)
        maps.append({
            "sT": sT, "kT": kT,
            "wq": tilew(Wq[sl, :].T),
            "wk": tilew(Wk[sl, :].T),
            "wv": tilew(Wv[sl, :].T),
            "wg": tilew(Wg[sl, :].T),
            "bq": np.ascontiguousarray(bqv[sl].reshape(CH, 1)),
            "mb": mb_t, "wo": tilew(woT),
        })
    return maps


def _run(inputs, trace=False):
    from concourse.bass_utils import run_bass_kernel_spmd

    nc = _get_nc(*_n2_for(inputs["mask"]))
    res = run_bass_kernel_spmd(nc, _in_maps(inputs),
                               core_ids=list(range(NCORES)), trace=trace)
    full = np.empty((S, C), dtype=np.float32)
    for c in range(NCORES):
        o = res.results[c]["out"]
        if NCOLL == 2:
            # core c computed rows {128c..} (part 0), {1024+128c..} (part 1)
            full[c * P:(c + 1) * P] = o[0:P]
            full[S // 2 + c * P:S // 2 + (c + 1) * P] = o[P:2 * P]
        else:
            full[c * SROWS:(c + 1) * SROWS] = o
    return full.reshape(B, S, C), res


def kernel(**inputs) -> np.ndarray:
    out, _ = _run(inputs, trace=False)
    return out
